# revision 17
# baseline (speedup 1.0000x reference)
"""Atomwise MLP + segment_sum kernel for 8 TRN2 NeuronCores.

Strategy (data-parallel over atoms, per sharding hint):
 - Host: shard x over 8 cores (125k atoms each, molecules contiguous since
   idx_m is sorted), pre-transpose each shard to feature-major [128, n] so
   the device DMAs are contiguous and matmuls need no on-device transpose.
 - Device (per core): tile over atoms; mm1 = W1^T-stationary matmul
   -> PSUM [64, T]; ScalarE silu(+b1) -> SBUF; mm2 = h-chunk-stationary
   matmul with W2 moving -> y_atom columns [128, 1]; batched lower-
   triangular matmul computes the inclusive prefix sum of y_atom within
   each 128-atom chunk; DMA the per-chunk prefixes out.
 - Host: segment sums are differences of the (chunk-offset-corrected)
   prefix at host-known segment boundaries; add b2 * segment counts.

No collectives needed: cores own disjoint atom ranges; boundary molecules
are summed on host when merging per-core partials.
"""

import numpy as np

N_CORES = 8
N_ATOMS = 1_000_000
N_PER_CORE = N_ATOMS // N_CORES  # 125_000
N_IN = 128
N_HID = 64
CHUNK = 128                      # atoms per y-column / prefix chunk
BLOCK_COLS = 128                 # chunks per prefix block (block = 16384 atoms)
MM_T = 512                       # atoms per mm1 matmul (one PSUM bank at f32 out)
SUPER_T = 1024                   # atoms per supertile (two mm1s packed on partitions)
N_PAD = 131_072                  # N_PER_CORE padded to a multiple of 16384
N_BLOCKS = N_PAD // (CHUNK * BLOCK_COLS)   # 8
G_TOTAL = N_PAD // CHUNK                   # 1024 chunks per core
SUPER_PER_BLOCK = (CHUNK * BLOCK_COLS) // SUPER_T  # 8
COLS_PER_SUPER = SUPER_T // CHUNK          # 16

_cached = {}
DTYPE = "bf16"  # active compute dtype for kernel()


DTYPES = {"f32": "float32", "bf16": "bfloat16", "fp8": "float8e4"}


def _build_nc(dtype="bf16", reps=1, ablate=()):
    from concourse import bacc, bass, mybir, tile

    dt_x = getattr(mybir.dt, DTYPES[dtype])     # x / W1 (mm1 operands)
    dt_h = mybir.dt.bfloat16 if dtype != "f32" else mybir.dt.float32  # hs / W2
    f32 = mybir.dt.float32

    nc = bacc.Bacc("TRN2", target_bir_lowering=False, debug=False)

    xT = nc.declare_dram_parameter("xT", [N_IN, N_PAD], dt_x, isOutput=False)
    w1 = nc.declare_dram_parameter("w1", [N_IN, N_HID], dt_x, isOutput=False)
    b1 = nc.declare_dram_parameter("b1", [CHUNK, 1], f32, isOutput=False)
    w2 = nc.declare_dram_parameter("w2", [CHUNK, 1], dt_h, isOutput=False)
    tri = nc.declare_dram_parameter("tri", [CHUNK, BLOCK_COLS], f32, isOutput=False)
    out = nc.declare_dram_parameter("out", [CHUNK, G_TOTAL], f32, isOutput=True)

    silu = mybir.ActivationFunctionType.Silu

    with tile.TileContext(nc) as tc:
        with (
            tc.tile_pool(name="const", bufs=1) as cpool,
            tc.tile_pool(name="x", bufs=4) as xpool,
            tc.tile_pool(name="h", bufs=3) as hpool,
            tc.tile_pool(name="y", bufs=2) as ypool,
            tc.tile_pool(name="po", bufs=2) as ppool,
            tc.tile_pool(name="ps_h", bufs=2, space=bass.MemorySpace.PSUM) as psh,
            tc.tile_pool(name="ps_y", bufs=2, space=bass.MemorySpace.PSUM) as psy,
            tc.tile_pool(name="ps_p", bufs=2, space=bass.MemorySpace.PSUM) as psp,
        ):
            w1_t = cpool.tile([N_IN, N_HID], dt_x)
            b1_t = cpool.tile([CHUNK, 1], f32)       # b1 duplicated on both halves
            w2_t = cpool.tile([CHUNK, 1], dt_h)      # W2 duplicated on both halves
            tri_t = cpool.tile([CHUNK, BLOCK_COLS], f32)
            nc.sync.dma_start(out=w1_t[:], in_=w1[:])
            nc.sync.dma_start(out=b1_t[:], in_=b1[:])
            nc.sync.dma_start(out=w2_t[:], in_=w2[:])
            nc.sync.dma_start(out=tri_t[:], in_=tri[:])

            xt_fixed = None
            if "dma" in ablate:
                xt_fixed = cpool.tile([N_IN, SUPER_T], dt_x)
                nc.sync.dma_start(out=xt_fixed[:], in_=xT[:, 0:SUPER_T])

            def body():
                for blk in range(N_BLOCKS):
                    y_mat = ypool.tile([CHUNK, BLOCK_COLS], f32)
                    for st in range(SUPER_PER_BLOCK):
                        g0 = blk * CHUNK * BLOCK_COLS + st * SUPER_T
                        if xt_fixed is None:
                            xt = xpool.tile([N_IN, SUPER_T], dt_x)
                            nc.sync.dma_start(out=xt[:], in_=xT[:, g0:g0 + SUPER_T])
                        else:
                            xt = xt_fixed
                        # two mm1s pack 2*MM_T atoms onto 128 psum partitions
                        hp = psh.tile([CHUNK, MM_T], f32)
                        if "mm1" in ablate:
                            nc.vector.memset(hp[:], 0.0)
                        else:
                            nc.tensor.matmul(hp[0:N_HID, :], w1_t[:], xt[:, 0:MM_T])
                            nc.tensor.matmul(hp[N_HID:CHUNK, :], w1_t[:],
                                             xt[:, MM_T:SUPER_T])
                        hs = hpool.tile([CHUNK, MM_T], dt_h)
                        if "act" in ablate:
                            nc.vector.tensor_copy(hs[:], hp[:])
                        else:
                            nc.scalar.activation(hs[:], hp[:], silu, bias=b1_t[:])
                        if "mm2" in ablate:
                            nc.vector.tensor_copy(
                                y_mat[:, st * COLS_PER_SUPER:(st + 1) * COLS_PER_SUPER],
                                hs[:, 0:COLS_PER_SUPER],
                            )
                        else:
                            yp = psy.tile([CHUNK, COLS_PER_SUPER], f32)
                            for c in range(COLS_PER_SUPER):
                                half = 0 if c < COLS_PER_SUPER // 2 else N_HID
                                cc = c % (COLS_PER_SUPER // 2)
                                nc.tensor.matmul(
                                    yp[:, c:c + 1],
                                    hs[half:half + N_HID,
                                       cc * CHUNK:(cc + 1) * CHUNK],
                                    w2_t[half:half + N_HID],
                                )
                            nc.vector.tensor_copy(
                                y_mat[:, st * COLS_PER_SUPER:(st + 1) * COLS_PER_SUPER],
                                yp[:],
                            )
                    pp = psp.tile([CHUNK, BLOCK_COLS], f32)
                    nc.tensor.matmul(pp[:], tri_t[:], y_mat[:])
                    po = ppool.tile([CHUNK, BLOCK_COLS], f32)
                    nc.vector.tensor_copy(po[:], pp[:])
                    nc.sync.dma_start(
                        out=out[:, blk * BLOCK_COLS:(blk + 1) * BLOCK_COLS],
                        in_=po[:],
                    )

            if reps == 1:
                body()
            else:
                with tc.For_i(0, reps, 1):
                    body()

    nc.compile()
    return nc


def _get_nc(dtype="bf16", reps=1, ablate=()):
    key = ("nc", dtype, reps, tuple(ablate))
    if key not in _cached:
        _cached[key] = _build_nc(dtype, reps, ablate)
    return _cached[key]


def build_in_maps(x, W1, b1, W2, dtype="bf16"):
    import ml_dtypes
    from concourse import mybir

    np_x = mybir.dt.np(getattr(mybir.dt, DTYPES[dtype]))
    np_h = ml_dtypes.bfloat16 if dtype != "f32" else np.float32
    tri_np = np.triu(np.ones((CHUNK, BLOCK_COLS), dtype=np.float32))
    w1_np = np.ascontiguousarray(W1, dtype=np.float32).astype(np_x)
    # b1 / W2 are duplicated onto both partition halves (see _build_nc)
    b1_half = np.asarray(b1, dtype=np.float32).reshape(N_HID, 1)
    b1_np = np.concatenate([b1_half, b1_half], axis=0)
    w2_half = np.asarray(W2, dtype=np.float32).reshape(N_HID, 1)
    w2_np = np.concatenate([w2_half, w2_half], axis=0).astype(np_h)

    in_maps = []
    for c in range(N_CORES):
        xs = x[c * N_PER_CORE:(c + 1) * N_PER_CORE]
        xt = np.zeros((N_IN, N_PAD), dtype=np_x)
        xt[:, :N_PER_CORE] = np.ascontiguousarray(xs.T).astype(np_x)
        in_maps.append({
            "xT": xt,
            "w1": w1_np,
            "b1": b1_np,
            "w2": w2_np,
            "tri": tri_np,
        })
    return in_maps


def run_device(x, W1, b1, W2, dtype="bf16", **run_kwargs):
    """Shard + run the NEFF on 8 cores; returns (per-core P arrays, results obj)."""
    from concourse.bass_utils import run_bass_kernel_spmd

    in_maps = build_in_maps(x, W1, b1, W2, dtype)
    nc = _get_nc(dtype)
    res = run_bass_kernel_spmd(nc, in_maps, core_ids=list(range(N_CORES)),
                               **run_kwargs)
    ps = [np.asarray(res.results[c]["out"], dtype=np.float32)
          for c in range(N_CORES)]
    return ps, res


def combine_host(ps, idx_m, num_segments, b2):
    """Per-core intra-chunk prefixes -> full segment sums."""
    nseg = int(num_segments)
    y = np.zeros(nseg, dtype=np.float64)
    idx_m = np.asarray(idx_m)
    for c in range(N_CORES):
        P = ps[c]  # [CHUNK, G_TOTAL]; column g = inclusive prefix of chunk g
        chunk_sums = P[CHUNK - 1, :].astype(np.float64)
        chunk_off = np.concatenate(([0.0], np.cumsum(chunk_sums)[:-1]))
        idx_c = idx_m[c * N_PER_CORE:(c + 1) * N_PER_CORE]
        mols, starts = np.unique(idx_c, return_index=True)
        ends = np.append(starts[1:], N_PER_CORE) - 1  # inclusive run ends

        def ploc(a):
            return P[a % CHUNK, a // CHUNK].astype(np.float64) + chunk_off[a // CHUNK]

        p_end = ploc(ends)
        s_safe = np.maximum(starts - 1, 0)
        p_start = np.where(starts > 0, ploc(s_safe), 0.0)
        np.add.at(y, mols, p_end - p_start)
    b2v = float(np.asarray(b2).reshape(-1)[0])
    if b2v != 0.0:
        y += np.bincount(idx_m, minlength=nseg).astype(np.float64) * b2v
    return y.astype(np.float32)


def kernel(x, W1, b1, W2, b2, idx_m, num_segments):
    x = np.asarray(x)
    ps, _ = run_device(x, W1, b1, W2, dtype=DTYPE)
    return combine_host(ps, idx_m, num_segments, b2)


# revision 22
# speedup vs baseline: 1.0949x; 1.0949x over previous
"""Atomwise MLP + segment_sum kernel for 8 TRN2 NeuronCores.

Strategy (data-parallel over atoms, per sharding hint):
 - Host: shard x over 8 cores (125k atoms each, molecules contiguous since
   idx_m is sorted), pre-transpose each shard to feature-major [128, n] so
   the device DMAs are contiguous and matmuls need no on-device transpose.
 - Device (per core): tile over atoms; mm1 = W1^T-stationary matmul
   -> PSUM [64, T]; ScalarE silu(+b1) -> SBUF; mm2 = h-chunk-stationary
   matmul with W2 moving -> y_atom columns [128, 1]; batched lower-
   triangular matmul computes the inclusive prefix sum of y_atom within
   each 128-atom chunk; DMA the per-chunk prefixes out.
 - Host: segment sums are differences of the (chunk-offset-corrected)
   prefix at host-known segment boundaries; add b2 * segment counts.

No collectives needed: cores own disjoint atom ranges; boundary molecules
are summed on host when merging per-core partials.
"""

import numpy as np

N_CORES = 8
N_ATOMS = 1_000_000
N_PER_CORE = N_ATOMS // N_CORES  # 125_000
N_IN = 128
N_HID = 64
CHUNK = 128                      # atoms per y-column / prefix chunk
BLOCK_COLS = 128                 # chunks per prefix block (block = 16384 atoms)
MM_T = 512                       # atoms per mm1 matmul (one PSUM bank at f32 out)
SUPER_T = 2048                   # atoms per supertile (4 mm1s into a 2-bank psum)
N_PAD = 131_072                  # N_PER_CORE padded to a multiple of 16384
N_BLOCKS = N_PAD // (CHUNK * BLOCK_COLS)   # 8
G_TOTAL = N_PAD // CHUNK                   # 1024 chunks per core
SUPER_PER_BLOCK = (CHUNK * BLOCK_COLS) // SUPER_T  # 8
COLS_PER_SUPER = SUPER_T // CHUNK          # 16

_cached = {}
DTYPE = "bf16"  # active compute dtype for kernel()


DTYPES = {"f32": "float32", "bf16": "bfloat16", "fp8": "float8e4"}


def _build_nc(dtype="bf16", reps=1, ablate=()):
    from concourse import bacc, bass, mybir, tile

    dt_x = getattr(mybir.dt, DTYPES[dtype])     # x / W1 (mm1 operands)
    dt_h = mybir.dt.bfloat16 if dtype != "f32" else mybir.dt.float32  # hs / W2
    f32 = mybir.dt.float32

    nc = bacc.Bacc("TRN2", target_bir_lowering=False, debug=False)

    xT = nc.declare_dram_parameter("xT", [N_IN, N_PAD], dt_x, isOutput=False)
    w1 = nc.declare_dram_parameter("w1", [N_IN, N_HID], dt_x, isOutput=False)
    b1 = nc.declare_dram_parameter("b1", [CHUNK, 1], f32, isOutput=False)
    w2 = nc.declare_dram_parameter("w2", [CHUNK, 1], dt_h, isOutput=False)
    tri = nc.declare_dram_parameter("tri", [CHUNK, BLOCK_COLS], f32, isOutput=False)
    out = nc.declare_dram_parameter("out", [CHUNK, G_TOTAL], f32, isOutput=True)

    silu = mybir.ActivationFunctionType.Silu

    with tile.TileContext(nc) as tc:
        with (
            tc.tile_pool(name="const", bufs=1) as cpool,
            tc.tile_pool(name="x", bufs=4) as xpool,
            tc.tile_pool(name="h", bufs=3) as hpool,
            tc.tile_pool(name="y", bufs=2) as ypool,
            tc.tile_pool(name="po", bufs=2) as ppool,
            tc.tile_pool(name="ps_h", bufs=2, space=bass.MemorySpace.PSUM) as psh,
            tc.tile_pool(name="ps_y", bufs=2, space=bass.MemorySpace.PSUM) as psy,
            tc.tile_pool(name="ps_p", bufs=2, space=bass.MemorySpace.PSUM) as psp,
        ):
            w1_t = cpool.tile([N_IN, N_HID], dt_x)
            b1_t = cpool.tile([CHUNK, 1], f32)       # b1 duplicated on both halves
            w2_t = cpool.tile([CHUNK, 1], dt_h)      # W2 duplicated on both halves
            tri_t = cpool.tile([CHUNK, BLOCK_COLS], f32)
            nc.sync.dma_start(out=w1_t[:], in_=w1[:])
            nc.sync.dma_start(out=b1_t[:], in_=b1[:])
            nc.sync.dma_start(out=w2_t[:], in_=w2[:])
            nc.sync.dma_start(out=tri_t[:], in_=tri[:])

            xt_fixed = None
            if "dma" in ablate:
                xt_fixed = cpool.tile([N_IN, SUPER_T], dt_x)
                nc.sync.dma_start(out=xt_fixed[:], in_=xT[:, 0:SUPER_T])

            SUPER_PER_DMA = 4            # 4 supertiles = 8192 atoms = 1MB at fp8

            def body():
                for blk in range(N_BLOCKS):
                    y_mat = ypool.tile([CHUNK, BLOCK_COLS], f32)
                    xt_big = None
                    for st in range(SUPER_PER_BLOCK):
                        g0 = blk * CHUNK * BLOCK_COLS + st * SUPER_T
                        if xt_fixed is None:
                            if st % SUPER_PER_DMA == 0:
                                xt_big = xpool.tile(
                                    [N_IN, SUPER_PER_DMA * SUPER_T], dt_x)
                                nc.sync.dma_start(
                                    out=xt_big[:],
                                    in_=xT[:, g0:g0 + SUPER_PER_DMA * SUPER_T])
                            off = (st % SUPER_PER_DMA) * SUPER_T
                            xt_src, xt_off = xt_big, off
                        else:
                            xt_src, xt_off = xt_fixed, 0
                        # 4 mm1s pack 4*MM_T atoms onto a [128, 1024] 2-bank psum
                        # group g (atoms g*512..): partitions (g%2)*64, free (g//2)*512
                        hp = psh.tile([CHUNK, 2 * MM_T], f32)
                        if "mm1" in ablate:
                            nc.vector.memset(hp[:], 0.0)
                        else:
                            for g in range(4):
                                p0 = (g % 2) * N_HID
                                f0 = (g // 2) * MM_T
                                a0 = xt_off + g * MM_T
                                nc.tensor.matmul(
                                    hp[p0:p0 + N_HID, f0:f0 + MM_T], w1_t[:],
                                    xt_src[:, a0:a0 + MM_T])
                        hs = hpool.tile([CHUNK, 2 * MM_T], dt_h)
                        if "act" in ablate:
                            nc.vector.tensor_copy(hs[:], hp[:])
                        else:
                            nc.scalar.activation(hs[:], hp[:], silu, bias=b1_t[:])
                        if "mm2" in ablate:
                            nc.vector.tensor_copy(
                                y_mat[:, st * COLS_PER_SUPER:(st + 1) * COLS_PER_SUPER],
                                hs[:, 0:COLS_PER_SUPER],
                            )
                        else:
                            yp = psy.tile([CHUNK, COLS_PER_SUPER], f32)
                            for c in range(COLS_PER_SUPER):
                                g = c // 4
                                cc = c % 4
                                p0 = (g % 2) * N_HID
                                f0 = (g // 2) * MM_T
                                nc.tensor.matmul(
                                    yp[:, c:c + 1],
                                    hs[p0:p0 + N_HID,
                                       f0 + cc * CHUNK:f0 + (cc + 1) * CHUNK],
                                    w2_t[p0:p0 + N_HID],
                                )
                            nc.vector.tensor_copy(
                                y_mat[:, st * COLS_PER_SUPER:(st + 1) * COLS_PER_SUPER],
                                yp[:],
                            )
                    pp = psp.tile([CHUNK, BLOCK_COLS], f32)
                    nc.tensor.matmul(pp[:], tri_t[:], y_mat[:])
                    po = ppool.tile([CHUNK, BLOCK_COLS], f32)
                    nc.vector.tensor_copy(po[:], pp[:])
                    nc.sync.dma_start(
                        out=out[:, blk * BLOCK_COLS:(blk + 1) * BLOCK_COLS],
                        in_=po[:],
                    )

            if reps == 1:
                body()
            else:
                with tc.For_i(0, reps, 1):
                    body()

    nc.compile()
    return nc


def _get_nc(dtype="bf16", reps=1, ablate=()):
    key = ("nc", dtype, reps, tuple(ablate))
    if key not in _cached:
        _cached[key] = _build_nc(dtype, reps, ablate)
    return _cached[key]


def build_in_maps(x, W1, b1, W2, dtype="bf16"):
    import ml_dtypes
    from concourse import mybir

    np_x = mybir.dt.np(getattr(mybir.dt, DTYPES[dtype]))
    np_h = ml_dtypes.bfloat16 if dtype != "f32" else np.float32
    tri_np = np.triu(np.ones((CHUNK, BLOCK_COLS), dtype=np.float32))
    w1_np = np.ascontiguousarray(W1, dtype=np.float32).astype(np_x)
    # b1 / W2 are duplicated onto both partition halves (see _build_nc)
    b1_half = np.asarray(b1, dtype=np.float32).reshape(N_HID, 1)
    b1_np = np.concatenate([b1_half, b1_half], axis=0)
    w2_half = np.asarray(W2, dtype=np.float32).reshape(N_HID, 1)
    w2_np = np.concatenate([w2_half, w2_half], axis=0).astype(np_h)

    in_maps = []
    for c in range(N_CORES):
        xs = x[c * N_PER_CORE:(c + 1) * N_PER_CORE]
        xt = np.zeros((N_IN, N_PAD), dtype=np_x)
        xt[:, :N_PER_CORE] = np.ascontiguousarray(xs.T).astype(np_x)
        in_maps.append({
            "xT": xt,
            "w1": w1_np,
            "b1": b1_np,
            "w2": w2_np,
            "tri": tri_np,
        })
    return in_maps


def run_device(x, W1, b1, W2, dtype="bf16", **run_kwargs):
    """Shard + run the NEFF on 8 cores; returns (per-core P arrays, results obj)."""
    from concourse.bass_utils import run_bass_kernel_spmd

    in_maps = build_in_maps(x, W1, b1, W2, dtype)
    nc = _get_nc(dtype)
    res = run_bass_kernel_spmd(nc, in_maps, core_ids=list(range(N_CORES)),
                               **run_kwargs)
    ps = [np.asarray(res.results[c]["out"], dtype=np.float32)
          for c in range(N_CORES)]
    return ps, res


def combine_host(ps, idx_m, num_segments, b2):
    """Per-core intra-chunk prefixes -> full segment sums."""
    nseg = int(num_segments)
    y = np.zeros(nseg, dtype=np.float64)
    idx_m = np.asarray(idx_m)
    for c in range(N_CORES):
        P = ps[c]  # [CHUNK, G_TOTAL]; column g = inclusive prefix of chunk g
        chunk_sums = P[CHUNK - 1, :].astype(np.float64)
        chunk_off = np.concatenate(([0.0], np.cumsum(chunk_sums)[:-1]))
        idx_c = idx_m[c * N_PER_CORE:(c + 1) * N_PER_CORE]
        mols, starts = np.unique(idx_c, return_index=True)
        ends = np.append(starts[1:], N_PER_CORE) - 1  # inclusive run ends

        def ploc(a):
            return P[a % CHUNK, a // CHUNK].astype(np.float64) + chunk_off[a // CHUNK]

        p_end = ploc(ends)
        s_safe = np.maximum(starts - 1, 0)
        p_start = np.where(starts > 0, ploc(s_safe), 0.0)
        np.add.at(y, mols, p_end - p_start)
    b2v = float(np.asarray(b2).reshape(-1)[0])
    if b2v != 0.0:
        y += np.bincount(idx_m, minlength=nseg).astype(np.float64) * b2v
    return y.astype(np.float32)


def kernel(x, W1, b1, W2, b2, idx_m, num_segments):
    x = np.asarray(x)
    ps, _ = run_device(x, W1, b1, W2, dtype=DTYPE)
    return combine_host(ps, idx_m, num_segments, b2)
